# revision 6
# baseline (speedup 1.0000x reference)
"""AttentionTopK (gated-attention MIL + top-k mean-pool + MLP) on 8 TRN2 cores.

Device kernel (data-parallel, 2 bags/core): streams x^T through two fused
float32r matmuls -> tanh/sigmoid gating -> wa-projection, producing raw
per-instance attention scores (the memory-bound part: 512 MB of x read once).
Host: softmax/mask/renorm, exact top-k (with fp32 re-scoring of a candidate
buffer so reduced-precision device scores can't flip the top-k set), feature
gather, mean-pool, classifier MLP.
"""
import numpy as np

import concourse.bass as bass
import concourse.tile as tile
from concourse import bacc, mybir
from concourse.bass_utils import run_bass_kernel_spmd

f32 = mybir.dt.float32
f32r = mybir.dt.float32r

# Problem shape (hardcoded per contract)
B, N, L, D = 16, 16384, 512, 128
NCORES = 8
BPC = B // NCORES            # bags per core
LCH = L // 128               # L chunks of 128 partitions
FCOL = 2048                  # columns fetched per DMA tile
SUB = 512                    # matmul moving free dim
NGRP = N // FCOL             # col groups per bag
NSUB = FCOL // SUB           # subgroups per group
EPS = 1e-8

_CACHE = {}


def _build():
    nc = bacc.Bacc("TRN2", target_bir_lowering=False, debug=False,
                   num_devices=NCORES)

    xt = nc.declare_dram_parameter("xt", [BPC, L, N], f32, isOutput=False)
    wv = nc.declare_dram_parameter("wv", [L, D], f32, isOutput=False)
    wu = nc.declare_dram_parameter("wu", [L, D], f32, isOutput=False)
    wa = nc.declare_dram_parameter("wa", [D, 1], f32, isOutput=False)
    bv = nc.declare_dram_parameter("bv", [D, 1], f32, isOutput=False)
    bu = nc.declare_dram_parameter("bu", [D, 1], f32, isOutput=False)
    out = nc.declare_dram_parameter("out", [BPC, N], f32, isOutput=True)

    TANH = mybir.ActivationFunctionType.Tanh
    SIG = mybir.ActivationFunctionType.Sigmoid

    with tile.TileContext(nc) as tc:
        with (
            tc.tile_pool(name="wpool", bufs=1) as wpool,
            tc.tile_pool(name="xpool", bufs=3) as xpool,
            tc.tile_pool(name="actpool", bufs=3) as actpool,
            tc.tile_pool(name="gpool", bufs=3) as gpool,
            tc.tile_pool(name="srow", bufs=2) as srowpool,
            tc.tile_pool(name="mm", bufs=2, space="PSUM") as mmpool,
            tc.tile_pool(name="sc", bufs=2, space="PSUM") as scpool,
        ):
            # ---- replicated params -> SBUF (fp32r where matmul operands) ----
            # weight chunk layout: w_sb[p, c*D + d] = W[c*128 + p, d]
            wv_sb = wpool.tile([128, LCH * D], f32r)
            nc.gpsimd.dma_start(
                wv_sb[:].rearrange("p (c d) -> p c d", c=LCH),
                wv[:, :].rearrange("(c p) d -> p c d", p=128),
            )
            wu_sb = wpool.tile([128, LCH * D], f32r)
            nc.gpsimd.dma_start(
                wu_sb[:].rearrange("p (c d) -> p c d", c=LCH),
                wu[:, :].rearrange("(c p) d -> p c d", p=128),
            )
            wa_sb = wpool.tile([D, 1], f32r)
            nc.gpsimd.dma_start(wa_sb[:], wa[:, :])
            bv_sb = wpool.tile([D, 1], f32)
            nc.sync.dma_start(bv_sb[:], bv[:, :])
            bu_sb = wpool.tile([D, 1], f32)
            nc.sync.dma_start(bu_sb[:], bu[:, :])

            for b in range(BPC):
                for g in range(NGRP):
                    srow = srowpool.tile([1, FCOL], f32)
                    # x^T tile: [p, c*FCOL + j] = xt[b, c*128+p, g*FCOL+j]
                    xtile = xpool.tile([128, LCH * FCOL], f32r)
                    nc.gpsimd.dma_start(
                        xtile[:].rearrange("p (c j) -> p c j", c=LCH),
                        xt[b].rearrange("(c p) n -> p c n", p=128)[
                            :, :, g * FCOL:(g + 1) * FCOL],
                    )
                    for sg in range(NSUB):
                        av = mmpool.tile([128, SUB], f32, tag="av")
                        au = mmpool.tile([128, SUB], f32, tag="au")
                        xs = xtile[:, :]  # helper slicing below
                        for c in range(LCH):
                            rhs = xtile[:, c * FCOL + sg * SUB:
                                        c * FCOL + (sg + 1) * SUB]
                            nc.tensor.matmul(
                                av[:], wv_sb[:, c * D:(c + 1) * D], rhs,
                                start=(c == 0), stop=(c == LCH - 1),
                            )
                        for c in range(LCH):
                            rhs = xtile[:, c * FCOL + sg * SUB:
                                        c * FCOL + (sg + 1) * SUB]
                            nc.tensor.matmul(
                                au[:], wu_sb[:, c * D:(c + 1) * D], rhs,
                                start=(c == 0), stop=(c == LCH - 1),
                            )
                        tav = actpool.tile([128, SUB], f32, tag="tav")
                        nc.scalar.activation(tav[:], av[:], TANH, bias=bv_sb[:])
                        sau = actpool.tile([128, SUB], f32, tag="sau")
                        nc.scalar.activation(sau[:], au[:], SIG, bias=bu_sb[:])
                        gt = gpool.tile([128, SUB], f32r)
                        nc.vector.tensor_mul(gt[:], tav[:], sau[:])
                        sps = scpool.tile([1, SUB], f32)
                        nc.tensor.matmul(sps[:], wa_sb[:], gt[:],
                                         start=True, stop=True)
                        dst = srow[0:1, sg * SUB:(sg + 1) * SUB]
                        if sg % 2 == 0:
                            nc.vector.tensor_copy(dst, sps[:])
                        else:
                            nc.scalar.copy(dst, sps[:])
                    nc.sync.dma_start(
                        out[b:b + 1, g * FCOL:(g + 1) * FCOL], srow[:])
    nc.compile()
    return nc


def _device_scores(x):
    """Raw attention scores [B, N] computed on 8 NeuronCores."""
    nc = _CACHE.get("nc")
    if nc is None:
        nc = _CACHE["nc"] = _build()
    w = _CACHE["weights"]
    xt = np.ascontiguousarray(x.transpose(0, 2, 1))  # [B, L, N]
    in_maps = []
    for c in range(NCORES):
        m = {"xt": xt[c * BPC:(c + 1) * BPC]}
        m.update(w)
        in_maps.append(m)
    res = run_bass_kernel_spmd(nc, in_maps, core_ids=list(range(NCORES)),
                               trace=_CACHE.get("trace", False),
                               **_CACHE.get("run_kwargs", {}))
    _CACHE["last_result"] = res
    s = np.empty((B, N), dtype=np.float32)
    for c in range(NCORES):
        s[c * BPC:(c + 1) * BPC] = res.results[c]["out"]
    return s


def _np_scores(rows, Wv, bv, Wu, bu, Wa, ba):
    """Exact fp32 reference scoring of selected rows [.., L] -> [..]."""
    av = np.tanh(rows @ Wv + bv)
    au = 1.0 / (1.0 + np.exp(-(rows @ Wu + bu)))
    return ((av * au) @ Wa + ba)[..., 0]


def kernel(x, mask, Wv, bv, Wu, bu, Wa, ba, W1, b1, W2, b2, top_k):
    x = np.ascontiguousarray(np.asarray(x, dtype=np.float32))
    mask = np.asarray(mask, dtype=np.float32)
    Wv = np.asarray(Wv, dtype=np.float32)
    bv = np.asarray(bv, dtype=np.float32)
    Wu = np.asarray(Wu, dtype=np.float32)
    bu = np.asarray(bu, dtype=np.float32)
    Wa = np.asarray(Wa, dtype=np.float32)
    ba = np.asarray(ba, dtype=np.float32)
    W1 = np.asarray(W1, dtype=np.float32)
    b1 = np.asarray(b1, dtype=np.float32)
    W2 = np.asarray(W2, dtype=np.float32)
    b2 = np.asarray(b2, dtype=np.float32)
    k = int(min(int(top_k), N))

    _CACHE["weights"] = {
        "wv": Wv, "wu": Wu, "wa": Wa,
        "bv": bv.reshape(D, 1), "bu": bu.reshape(D, 1),
    }

    s = _device_scores(x)                       # [B, N] raw scores
    s = s + ba.reshape(1,)[0]                   # K=1 scalar bias

    # softmax over instances (matches jax.nn.softmax in fp32)
    m = s.max(axis=1, keepdims=True)
    e = np.exp(s - m)
    A = e / e.sum(axis=1, keepdims=True)
    A = A * mask
    A = A / (A.sum(axis=1, keepdims=True) + np.float32(EPS))
    A = A.astype(np.float32)

    # top-k set: pick candidates from device scores, re-score exactly on host
    cbuf = min(N, k + 30)
    cand = np.argpartition(-A, cbuf - 1, axis=1)[:, :cbuf]        # [B, C]
    crows = x[np.arange(B)[:, None], cand]                        # [B, C, L]
    s_ex = _np_scores(crows, Wv, bv, Wu, bu, Wa, ba)              # [B, C]
    if mask is not None:
        cmask = mask[np.arange(B)[:, None], cand]
        # masked-out entries have A=0; order them below all unmasked ones
        s_ex = np.where(cmask > 0, s_ex, -np.inf)
    # order like jax.lax.top_k: by value desc, ties by index asc
    order = np.lexsort((cand, -s_ex), axis=1)[:, :k]
    topk_idx = np.take_along_axis(cand, order, axis=1)            # [B, k]

    feats = x[np.arange(B)[:, None], topk_idx]                    # [B, k, L]
    pooled = feats.mean(axis=1)                                   # [B, L]
    h = np.maximum(pooled @ W1 + b1, 0.0)
    Y_prob = (h @ W2 + b2).astype(np.float32)
    Y_hat = np.argmax(Y_prob, axis=1).astype(np.int32)
    return Y_prob, Y_hat, A
